# revision 45
# baseline (speedup 1.0000x reference)
"""Trainium2 Bass kernel for nn_MultiHeadAttention_66984309948505.

Full causal MHA: x[4,2048,1024], 16 heads of 64, out-proj + bias.

Sharding (8 cores): 4-way data-parallel over batch x 2-way tensor-parallel
over heads. Core (b, g) computes heads [8g, 8g+8) for batch b, including the
partial output projection Y_partial = O_g @ Wo[:, 512g:512(g+1)].T.
Host-side unshard: Y[b] = (Y_partial[b,g=0] + Y_partial[b,g=1]).T + bo.

v2 design (vs the fp32r v1 baseline):
  - all matmul operands in bf16 (rel-err ~4e-3 vs the 2e-2 gate); PSUM f32.
  - causal diag-block mask via gpsimd affine_select on pt (was DVE tri-mul).
  - softmax normalization: DVE reciprocal -> gpsimd partition_broadcast ->
    one DVE mul per head; norm emission deferred past the next block's
    projection copies so those copies aren't stuck behind it in the DVE FIFO.
  - weights/x loaded via coalesced DMAs, ordered so the first projection
    (wq stack 0 + x chunk 0) lands as early as possible.
  - fine-grained PE scheduling: QKV projections of chunk c+1 and the output
    projection of chunk c-1 are cut into single-matmul "supply units" pumped
    between attention kt steps, so the PE never starves while the scalar
    engine (softmax exp, the in-loop bottleneck) catches up. Out-proj units
    live in a second queue interleaved 1:2 so their PSUM-slot WAR chains
    don't head-of-line-block the PE during marker drains.
  - chunk-2/chunk-3 attention blocks interleaved so the exp-heavy chunk 3
    is not an ACT-bound tail.
  - AV emission software-pipelined 2 kt deep (exp+mask latency hiding).
  - y staged per chunk in one [P, ND, TCH] tile, written with 4 paired-et
    DMAs; final chunk's PSUM->SBUF copies alternate DVE/ACT to halve the
    end-of-kernel tail.

Device layouts all feature-major so no on-chip transposes are needed:
  xt [c, p, dt, t], QT/KT [S, t] (head h at partitions (h%2)*64 of stack
  h//2), scores S^T[k, q], softmax denominator free via a ones-column
  appended to V (PSUM row 64), O^T [S, t], Y^T [D, t].
"""

import numpy as np
import ml_dtypes

import concourse.bacc as bacc
import concourse.bass as bass
import concourse.mybir as mybir
import concourse.tile as tile
from concourse.bass_utils import run_bass_kernel_spmd

# Problem constants (hardcoded per contract)
B, T, D = 4, 2048, 1024
H, HS = 16, 64
NCORES = 8
HG = 2                 # head-group TP degree
H_LOC = H // HG        # 8 heads per core
S = H_LOC * HS         # 512 local head dims
P = 128
TCH = 512              # t/q chunk width
NCHUNK = T // TCH      # 4
ND = D // P            # 8 d-tiles
NSP = S // P           # 4 head stacks
NTT = TCH // P         # 4 k-subtiles per chunk
SCALE = 1.0 / np.sqrt(HS)

F32 = mybir.dt.float32
BF16 = mybir.dt.bfloat16
EXP = mybir.ActivationFunctionType.Exp
CPY = mybir.ActivationFunctionType.Copy
GE = mybir.AluOpType.is_ge

BLOCKS = [(0, 0), (0, 1), (0, 2), (0, 3),
          (1, 0), (1, 1), (1, 2), (1, 3),
          (2, 0), (3, 0), (2, 1), (3, 1), (2, 2), (3, 2), (2, 3), (3, 3)]


def build_program(reps: int = 1, mask: str = "dve"):
    nc = bacc.Bacc("TRN2", target_bir_lowering=False, debug=False)

    xt = nc.dram_tensor("xt", [NCHUNK, P, 2, ND // 2, TCH], BF16,
                        kind="ExternalInput")
    wq = nc.dram_tensor("wq", [P, NSP, ND, P], BF16, kind="ExternalInput")
    wk = nc.dram_tensor("wk", [P, NSP, ND, P], BF16, kind="ExternalInput")
    wv = nc.dram_tensor("wv", [P, ND, S], BF16, kind="ExternalInput")
    wot = nc.dram_tensor("wot", [P, NSP, D], BF16, kind="ExternalInput")
    tri = nc.dram_tensor("tri", [P, P], BF16, kind="ExternalInput")
    yt = nc.dram_tensor("yt", [NCHUNK, P, ND, TCH], BF16,
                        kind="ExternalOutput")

    with tile.TileContext(nc) as tc:
        with (
            nc.allow_low_precision(reason="bf16 matmul operands, fp32 accum"),
            tc.tile_pool(name="const", bufs=1) as constp,
            tc.tile_pool(name="kv", bufs=1) as kvp,
            tc.tile_pool(name="qt", bufs=2) as qtp,
            tc.tile_pool(name="osb", bufs=4) as osbp,
            tc.tile_pool(name="xp", bufs=2) as xp,
            tc.tile_pool(name="ptp", bufs=4) as ptp,
            tc.tile_pool(name="rcpp", bufs=2) as rcpp,
            tc.tile_pool(name="bcp", bufs=2) as bcp,
            tc.tile_pool(name="stg", bufs=2) as stp,
            tc.tile_pool(name="psS", bufs=2, space="PSUM") as psS,
            tc.tile_pool(name="psACC", bufs=4, space="PSUM") as psACC,
        ):
            # Resident weights. DMA order is chosen so the first projection
            # work (wq stack 0, x chunk 0) is available as early as possible
            # (the cost-model DMA device serializes transfers in issue order).
            wq_sb = constp.tile([P, NSP, ND, P], BF16, name="wq_sb")
            wk_sb = constp.tile([P, NSP, ND, P], BF16, name="wk_sb")
            wv_sb = constp.tile([P, ND, S], BF16, name="wv_sb")
            wot_sb = constp.tile([P, NSP, D], BF16, name="wot_sb")
            xt0_tiles = {}
            if reps == 1:
                # startup pipeline: smallest critical pieces first so the
                # first projection matmuls start as early as possible, with
                # the later halves landing while the PE chews the first ones
                xt0_tiles[0] = xp.tile([P, 2, ND // 2, TCH], BF16, tag="x",
                                       name="x0")
                nc.sync.dma_start(out=wq_sb[:, 0, 0:4], in_=wq[:, 0, 0:4])
                nc.sync.dma_start(out=xt0_tiles[0][:, 0, 0:2],
                                  in_=xt[0, :, 0, 0:2])
                nc.sync.dma_start(out=xt0_tiles[0][:, 0, 2:4],
                                  in_=xt[0, :, 0, 2:4])
                nc.sync.dma_start(out=wq_sb[:, 0, 4:8], in_=wq[:, 0, 4:8])
                nc.sync.dma_start(out=wk_sb[:, 0, 0:4], in_=wk[:, 0, 0:4])
                nc.sync.dma_start(out=xt0_tiles[0][:, 1], in_=xt[0, :, 1])
                nc.sync.dma_start(out=wk_sb[:, 0, 4:8], in_=wk[:, 0, 4:8])
            else:
                nc.sync.dma_start(out=wq_sb[:, 0], in_=wq[:, 0])
                nc.sync.dma_start(out=wk_sb[:, 0], in_=wk[:, 0])
            nc.sync.dma_start(out=wv_sb[:, 0:4], in_=wv[:, 0:4])
            nc.sync.dma_start(out=wv_sb[:, 4:8], in_=wv[:, 4:8])
            for st in range(1, NSP):
                nc.sync.dma_start(out=wq_sb[:, st], in_=wq[:, st])
                nc.sync.dma_start(out=wk_sb[:, st], in_=wk[:, st])
            nc.sync.dma_start(out=wot_sb[:], in_=wot[:])
            tri_sb = constp.tile([P, P], BF16, name="tri_sb")
            if mask in ("dve", "hybrid"):
                nc.sync.dma_start(out=tri_sb[:], in_=tri[:])

            # Resident K^T and V (per-chunk tiles for clean dep tracking).
            kt_sb = [kvp.tile([P, NSP, TCH], BF16, name=f"kt{c}")
                     for c in range(NCHUNK)]
            v_sb = [kvp.tile([P, NTT, H_LOC, HS + 1], BF16, name=f"v{c}")
                    for c in range(NCHUNK)]

            def emit_body():
                supply = []      # projection units (markers index into it)
                op_supply = []   # out-projection units, interleaved 1:2
                pos = [0]
                toggle = [0]
                markers = {}
                qt_tiles = {}
                xt_tiles = {}
                o_sb_tiles = {}
                y_stage = {}

                def pump_one():
                    # projection units first: out-proj units are the only
                    # fill available late in the schedule (the ACT-bound
                    # chunk-2/3 region), so save them for when the
                    # projection supply has run dry
                    if pos[0] < len(supply):
                        supply[pos[0]]()
                        pos[0] += 1
                        return True
                    if op_supply:
                        op_supply.pop(0)()
                        return True
                    return False

                def pump(n=1):
                    for _ in range(n):
                        if not pump_one():
                            break

                def pump_until(idx):
                    while pos[0] < idx:
                        supply[pos[0]]()
                        pos[0] += 1

                def x_ap(c, dt):
                    return xt_tiles[c][:, dt // (ND // 2), dt % (ND // 2), :]

                def proj_units(c, st, which):
                    """8 matmuls + 1 copy producing qt/kt stack `st`."""
                    hold = {}

                    def mk(dt):
                        def u():
                            if dt == 0:
                                hold["ps"] = psACC.tile([P, TCH], F32,
                                                        tag="acc", name="prj")
                            w_sb = wq_sb if which == "q" else wk_sb
                            nc.tensor.matmul(
                                hold["ps"][:], w_sb[:, st, dt, :],
                                x_ap(c, dt),
                                start=(dt == 0), stop=(dt == ND - 1))
                        return u

                    def cp():
                        dst = qt_tiles[c] if which == "q" else kt_sb[c]
                        nc.vector.tensor_copy(dst[:, st, :], hold["ps"][:])

                    return [mk(dt) for dt in range(ND)] + [cp]

                def v_units(c, tt):
                    hold = {}

                    def mk(dt):
                        def u():
                            if dt == 0:
                                hold["ps"] = psACC.tile([P, TCH], F32,
                                                        tag="acc", name="vprj")
                            nc.tensor.matmul(
                                hold["ps"][:],
                                x_ap(c, dt)[:, tt * P:(tt + 1) * P],
                                wv_sb[:, dt, :],
                                start=(dt == 0), stop=(dt == ND - 1))
                        return u

                    def cp():
                        nc.vector.tensor_copy(
                            v_sb[c][:, tt, :, 0:HS],
                            hold["ps"][:].rearrange("p (h e) -> p h e",
                                                    h=H_LOC))

                    return [mk(dt) for dt in range(ND)] + [cp]

                def enqueue_proj(c):
                    if c in xt0_tiles:
                        xt_tiles[c] = xt0_tiles[c]
                    else:
                        xt_tiles[c] = xp.tile([P, 2, ND // 2, TCH], BF16,
                                              tag="x", name=f"x{c}")
                        nc.sync.dma_start(out=xt_tiles[c][:, 0],
                                          in_=xt[c, :, 0])
                        nc.sync.dma_start(out=xt_tiles[c][:, 1],
                                          in_=xt[c, :, 1])
                    nc.gpsimd.memset(v_sb[c][:, :, :, HS:HS + 1], 1.0)
                    qt_tiles[c] = qtp.tile([P, NSP, TCH], BF16, tag="qt",
                                           name=f"qt{c}")
                    if c == 0:
                        # interleave q/k stack-0 at dt granularity: the PE
                        # queue is in-order and x arrives in two half-DMAs,
                        # so q dts 4-7 must not block k dts 0-3 at startup
                        qu = proj_units(0, 0, "q")
                        ku = proj_units(0, 0, "k")
                        supply.extend(qu[0:4] + ku[0:4] + qu[4:8] + ku[4:8]
                                      + [qu[8], ku[8]])
                    else:
                        supply.extend(proj_units(c, 0, "q"))
                        supply.extend(proj_units(c, 0, "k"))
                    for tt in range(NTT):
                        supply.extend(v_units(c, tt))
                    markers[(c, 0)] = len(supply)
                    for st in range(1, NSP):
                        supply.extend(proj_units(c, st, "q"))
                        supply.extend(proj_units(c, st, "k"))
                        markers[(c, st)] = len(supply)

                def outproj_units(c):
                    # y for chunk c staged in one [P, ND, TCH] tile; DMA in
                    # et pairs so transfers overlap the remaining copies.
                    use_act = (c == NCHUNK - 1)
                    stg = stp.tile([P, ND, TCH], BF16, tag="y", name=f"y{c}")
                    y_stage[c] = stg
                    units = []
                    for et in range(ND):
                        hold = {}

                        def mk(sp, et=et, hold=hold):
                            def u():
                                if sp == 0:
                                    hold["ps"] = psACC.tile(
                                        [P, TCH], F32, tag="acc", name="yps")
                                nc.tensor.matmul(
                                    hold["ps"][:],
                                    wot_sb[:, sp, et * P:(et + 1) * P],
                                    o_sb_tiles[c][:, sp, :],
                                    start=(sp == 0), stop=(sp == NSP - 1))
                            return u

                        def cp(et=et, hold=hold):
                            if use_act and et % 2 == 1:
                                nc.scalar.activation(stg[:, et, :],
                                                     hold["ps"][:], CPY)
                            else:
                                nc.vector.tensor_copy(stg[:, et, :],
                                                      hold["ps"][:])

                        def dm(et=et):
                            nc.sync.dma_start(
                                out=yt[c, :, et - 1:et + 1, :],
                                in_=stg[:, et - 1:et + 1, :])

                        def dm1(et=et):
                            nc.sync.dma_start(out=yt[c, :, et:et + 1, :],
                                              in_=stg[:, et:et + 1, :])

                        tail = []
                        if use_act and et >= ND - 2:
                            tail = [dm1]  # singles shorten the end tail
                        elif et % 2 == 1:
                            tail = [dm]
                        units.append(([mk(sp) for sp in range(NSP)], cp, tail))
                    if not use_act:
                        return [u for ms, cp_, tl in units
                                for u in ms + [cp_] + tl]
                    # final chunk: the first group's sp3 matmul waits on the
                    # last block's norm; pre-issue ets 0-1's norm-independent
                    # sp0-2 ahead of it so the in-order PE queue isn't
                    # head-of-line blocked during the norm chain (only 2
                    # accumulation groups fit the free PSUM slots)
                    flat = (units[0][0][0:3] + units[1][0][0:3]
                            + [units[0][0][3], units[0][1],
                               units[1][0][3], units[1][1]] + units[1][2])
                    for ms, cp_, tl in units[2:]:
                        flat.extend(ms + [cp_] + tl)
                    return flat

                def emit_block(c, hp):
                    """Attention for (chunk c, head-pair hp). Returns the
                    deferred normalization closure."""
                    qt_c = qt_tiles[c]
                    nkt = 4 * c + 4
                    o_ps = [psACC.tile([P, TCH], F32, tag="acc", name="o_ps")
                            for _ in range(2)]

                    def emit_av(kt, pt, q0):
                        cc, tt = kt // 4, kt % 4
                        for j in range(2):
                            nc.tensor.matmul(
                                o_ps[j][0:HS + 1, q0:],
                                v_sb[cc][:, tt, 2 * hp + j, :],
                                pt[:, j, q0:],
                                start=(kt == 0), stop=(kt == nkt - 1))

                    pend = []
                    for kt in range(nkt):
                        cc, tt = kt // 4, kt % 4
                        q0 = max(0, P * kt - TCH * c)
                        s_ps = psS.tile([P, 2, TCH], F32, tag="s", name="s_ps")
                        for j in range(2):
                            rows = slice(j * 64, j * 64 + 64)
                            nc.tensor.matmul(
                                s_ps[:, j, q0:],
                                kt_sb[cc][rows, hp, tt * P:(tt + 1) * P],
                                qt_c[rows, hp, q0:],
                                start=True, stop=True)
                        pt = ptp.tile([P, 2, TCH], BF16, tag="pt", name="pt")
                        nc.scalar.activation(
                            pt[:, :, q0:], s_ps[:, :, q0:], EXP,
                            scale=float(SCALE))
                        if kt >= 4 * c:  # diagonal block: causal mask
                            # chunks 0-1: DVE is congested with projection
                            # copies, so mask on the idle Pool engine there;
                            # later chunks: DVE tri-mul (shorter chain)
                            on_pool = (mask == "pool" or
                                       (mask == "hybrid" and c <= 1))
                            for j in range(2):
                                if on_pool:
                                    nc.gpsimd.affine_select(
                                        pt[:, j, q0:q0 + P],
                                        pt[:, j, q0:q0 + P],
                                        pattern=[[1, P]], compare_op=GE,
                                        fill=0.0, base=0,
                                        channel_multiplier=-1)
                                else:
                                    nc.vector.tensor_mul(
                                        pt[:, j, q0:q0 + P],
                                        pt[:, j, q0:q0 + P], tri_sb[:])
                        if len(pend) == 3:
                            emit_av(*pend.pop(0))
                        pump(1)
                        pend.append((kt, pt, q0))
                    # trailing AVs: no next-kt scores to hide exp+mask
                    # latency behind, so feed the PE supply units instead
                    while pend:
                        emit_av(*pend.pop(0))
                        pump(4 if pend else 2)

                    def norm(filler=None):
                        # rows 0:64 of o_ps[j] / row 64 (ones-column rowsum).
                        # partition_broadcast requires out base partition 0,
                        # so both head halves ride the free axis.
                        rcp2 = rcpp.tile([1, 2, TCH], BF16, tag="rcp",
                                         name="rcp2")
                        bc = bcp.tile([64, 2, TCH], BF16, tag="bc",
                                      name="bc_sb")
                        for j in range(2):
                            nc.vector.reciprocal(rcp2[0:1, j, :],
                                                 o_ps[j][64:65, :])
                        for j in range(2):
                            nc.gpsimd.partition_broadcast(
                                bc[0:64, j, :], rcp2[0:1, j, :], channels=64)
                        if filler is not None:
                            # emitted between the Pool broadcast and the DVE
                            # muls: these units' copies enter the DVE FIFO
                            # ahead of the muls, freeing their PSUM slots
                            # while the broadcast runs
                            filler()
                        for j in range(2):
                            nc.vector.tensor_mul(
                                o_sb_tiles[c][j * 64:(j + 1) * 64, hp, :],
                                o_ps[j][0:64, :], bc[0:64, j, :])

                    return norm

                enqueue_proj(0)
                done = {c: 0 for c in range(NCHUNK)}
                norm_prev = None
                for i, (c, hp) in enumerate(BLOCKS):
                    if hp == 0:
                        if c + 1 < NCHUNK:
                            enqueue_proj(c + 1)
                        o_sb_tiles[c] = osbp.tile([P, NSP, TCH], BF16,
                                                  tag="o", name=f"o{c}")
                    # drain one block AHEAD so the next block's projection
                    # copies aren't just-in-time, and ahead of the previous
                    # block's norm chain in the DVE queue
                    nxt = BLOCKS[min(i + 1, len(BLOCKS) - 1)]
                    pump_until(max(markers[(c, hp)], markers[nxt]))
                    if norm_prev is not None:
                        norm_prev()
                    norm_prev = emit_block(c, hp)
                    done[c] += 1
                    if done[c] == 4 and c < NCHUNK - 1:
                        # out-proj units read o_sb, so the last block's norm
                        # must be emitted before they can enter the pump queue
                        norm_prev()
                        norm_prev = None
                        op_supply.extend(outproj_units(c))
                if norm_prev is not None:
                    def _endfill():
                        for _ in range(6):
                            if op_supply:
                                op_supply.pop(0)()
                    norm_prev(_endfill)
                pump_until(len(supply))
                # leftover out-proj units interleave 1:1 into the final
                # chunk's out-projection: they fill the PE while the last
                # block's norm chain (recip -> broadcast -> mul) completes,
                # and are exhausted well before the final y DMAs
                for u in outproj_units(NCHUNK - 1):
                    u()
                    if op_supply:
                        op_supply.pop(0)()
                while op_supply:
                    op_supply.pop(0)()

            import contextlib
            loop_ctx = (tc.For_i(0, reps, 1) if reps > 1
                        else contextlib.nullcontext())
            with loop_ctx:
                emit_body()

    nc.compile()
    return nc


_CACHE = {}


def _get_program(reps: int = 1, mask: str = "dve"):
    key = ("nc", reps, mask)
    if key not in _CACHE:
        _CACHE[key] = build_program(reps, mask)
    return _CACHE[key]


def make_in_maps(x, Wq, Wk, Wv, Wo):
    bf = ml_dtypes.bfloat16
    x = np.asarray(x, dtype=np.float32)
    Wq = np.asarray(Wq, dtype=np.float32)
    Wk = np.asarray(Wk, dtype=np.float32)
    Wv = np.asarray(Wv, dtype=np.float32)
    Wo = np.asarray(Wo, dtype=np.float32)
    tri = np.triu(np.ones((P, P), dtype=np.float32)).astype(bf)

    def wmat_qk(W, g):
        # [H_LOC, D, HS] -> [D, S] -> [P, NSP, ND, P] (st-major for split DMA)
        m = W[g * H_LOC:(g + 1) * H_LOC].transpose(1, 0, 2).reshape(D, S)
        return np.ascontiguousarray(
            m.reshape(ND, P, NSP, P).transpose(1, 2, 0, 3))

    def wmat_v(W, g):
        m = W[g * H_LOC:(g + 1) * H_LOC].transpose(1, 0, 2).reshape(D, S)
        return np.ascontiguousarray(m.reshape(ND, P, S).transpose(1, 0, 2))

    in_maps = []
    for core in range(NCORES):
        b, g = core // HG, core % HG
        xT = x[b].T  # [D, T]
        xt_t = np.ascontiguousarray(
            xT.reshape(2, ND // 2, P, NCHUNK, TCH).transpose(3, 2, 0, 1, 4))
        woT = Wo[:, g * S:(g + 1) * S].T  # [S, D]
        wot_t = np.ascontiguousarray(woT.reshape(NSP, P, D).transpose(1, 0, 2))
        in_maps.append({
            "xt": xt_t.astype(bf),
            "wq": wmat_qk(Wq, g).astype(bf),
            "wk": wmat_qk(Wk, g).astype(bf),
            "wv": wmat_v(Wv, g).astype(bf),
            "wot": wot_t.astype(bf),
            "tri": tri,
        })
    return in_maps


def kernel_ex(x, Wq, Wk, Wv, Wo, bo, **run_kwargs):
    """Run and return (output, BassKernelResults)."""
    nc = _get_program()
    in_maps = make_in_maps(x, Wq, Wk, Wv, Wo)
    res = run_bass_kernel_spmd(nc, in_maps, core_ids=list(range(NCORES)),
                               **run_kwargs)
    bo = np.asarray(bo, dtype=np.float32)
    y = np.empty((B, T, D), dtype=np.float32)
    for b in range(B):
        # yt [NCHUNK, P, ND, TCH] -> Y^T [D, T]: row et*P+p, col c*TCH+t
        yts = [res.results[HG * b + g]["yt"].astype(np.float32)
               for g in range(HG)]
        ysum = yts[0] + yts[1]
        y[b] = ysum.transpose(2, 1, 0, 3).reshape(D, T).T + bo
    return y, res


def kernel(x, Wq, Wk, Wv, Wo, bo):
    y, _ = kernel_ex(x, Wq, Wk, Wv, Wo, bo)
    return y
